# revision 41
# baseline (speedup 1.0000x reference)
"""LlamaAttention (B=2,S=2048,H=4096, 32 q heads / 8 kv heads, RoPE, causal)
on 8 trn2 cores. Sharding: DP=2 over batch x TP=4 over heads.

v18: v16 + cos/sin DMA ahead of the x block, projection order K0 -> V
-> K1 (V proj no longer serialized behind both K ropes), Wo strip
prefetch deepened to og+3.
"""
import sys
if "/opt/trn_rl_repo" not in sys.path:
    sys.path.insert(0, "/opt/trn_rl_repo")

import numpy as np
import ml_dtypes

S = 2048
H = 4096
HD = 128
NHL = 8        # q heads per core
NKVL = 2       # kv heads per core
QF = NHL * HD  # 1024
KF = NKVL * HD  # 256
TB = 512       # token block
NTB = S // TB  # 4
KB = H // 128  # 32 contraction tiles for projections

_CACHE = {}
LAST = {}


def _build():
    if "nc" in _CACHE:
        return _CACHE["nc"]
    import concourse.bacc as bacc
    import concourse.mybir as mybir
    from concourse.tile import TileContext

    F32 = mybir.dt.float32
    BF16 = mybir.dt.bfloat16
    EXP = mybir.ActivationFunctionType.Exp
    COPYF = mybir.ActivationFunctionType.Copy
    SCALE = 1.0 / float(np.sqrt(HD))

    _ctr = [0]

    def _nm(p):
        _ctr[0] += 1
        return f"{p}{_ctr[0]}"

    nc = bacc.Bacc("TRN2", target_bir_lowering=False, debug=False, num_devices=8)
    xt = nc.declare_dram_parameter("xt", [H, S], BF16, isOutput=False)
    wqt = nc.declare_dram_parameter("wqt", [H, QF], BF16, isOutput=False)
    wkt = nc.declare_dram_parameter("wkt", [H, KF], BF16, isOutput=False)
    wvt = nc.declare_dram_parameter("wvt", [H, KF], BF16, isOutput=False)
    wot = nc.declare_dram_parameter("wot", [QF, H], BF16, isOutput=False)
    cs = nc.declare_dram_parameter("cs", [128, S], BF16, isOutput=False)
    sn = nc.declare_dram_parameter("sn", [128, S], BF16, isOutput=False)
    msk = nc.declare_dram_parameter("msk", [128, 128], BF16, isOutput=False)
    idn = nc.declare_dram_parameter("idn", [128, 128], BF16, isOutput=False)
    out_t = nc.declare_dram_parameter("out_t", [H, S], BF16, isOutput=True)

    wqt_r = wqt.rearrange("(kb p) m -> p kb m", p=128)
    wkt_r = wkt.rearrange("(kb p) m -> p kb m", p=128)
    wvt_r = wvt.rearrange("(kb p) m -> p kb m", p=128)
    wot_r = wot.rearrange("(kb p) m -> p kb m", p=128)
    xt_r = xt.rearrange("(kg b p) t -> p kg b t", p=128, b=4)  # [128, 8, 4, S]
    out_r = out_t.rearrange("(og b p) t -> p og b t", p=128, b=2)  # [128, 16, 2, S]

    from contextlib import ExitStack

    with ExitStack() as ctx:
        tc = ctx.enter_context(TileContext(nc))
        pc = ctx.enter_context(tc.tile_pool(name="const", bufs=1))
        px = ctx.enter_context(tc.tile_pool(name="xx", bufs=9))
        pxs = ctx.enter_context(tc.tile_pool(name="xs", bufs=4))
        pwqk = ctx.enter_context(tc.tile_pool(name="wqk", bufs=2))
        pkw = ctx.enter_context(tc.tile_pool(name="kw", bufs=1))
        pwv = ctx.enter_context(tc.tile_pool(name="wv", bufs=1))
        pq = ctx.enter_context(tc.tile_pool(name="qt", bufs=10))
        pk = ctx.enter_context(tc.tile_pool(name="kt", bufs=2))
        pv = ctx.enter_context(tc.tile_pool(name="vv", bufs=16))
        pa = ctx.enter_context(tc.tile_pool(name="at", bufs=16))
        pp = ctx.enter_context(tc.tile_pool(name="pt", bufs=26))
        pasb = ctx.enter_context(tc.tile_pool(name="asb", bufs=10))
        pcs = ctx.enter_context(tc.tile_pool(name="csn", bufs=2))
        pr = ctx.enter_context(tc.tile_pool(name="rope", bufs=2))
        psmall = ctx.enter_context(tc.tile_pool(name="sm", bufs=8))
        pwo = ctx.enter_context(tc.tile_pool(name="wo", bufs=4))
        pob = ctx.enter_context(tc.tile_pool(name="ob", bufs=3))
        psA = ctx.enter_context(tc.tile_pool(name="psA", bufs=2, space="PSUM"))
        psS = ctx.enter_context(tc.tile_pool(name="psS", bufs=3, space="PSUM"))
        psO = ctx.enter_context(tc.tile_pool(name="psO", bufs=3, space="PSUM"))
        if True:
            msk_sb = pc.tile([128, 128], BF16, tag="msk")
            nc.sync.dma_start(out=msk_sb[:], in_=msk[:])
            idn_sb = pc.tile([128, 128], BF16, tag="idn")
            nc.sync.dma_start(out=idn_sb[:], in_=idn[:])

            # HAM warmup: dependency-free matmuls on the identity tile keep
            # the PE busy through a SHORT window so real chains start at 2.4G.
            for _ in range(28):
                w = psS.tile([128, 512], F32, tag="st", name=_nm("warm"))
                nc.tensor.matmul(w[:, 0:128], idn_sb[:], idn_sb[:],
                                 start=True, stop=True)

            # persistent K^T [hd, S] per kv head, and V [tok, (kv, hd|1)]
            kts = [pk.tile([128, S], BF16, tag="kt", name=f"ktp{i}") for i in range(NKVL)]
            vts = []  # 16 tiles [128, NKVL, 129]
            kwt = None
            vstrip = None

            def rope(dst, ps, cs_t, sn_t):
                tmp = pr.tile([128, TB], BF16, tag="rsin", name=_nm("rsin"))
                nc.vector.tensor_mul(tmp[0:64, :], ps[64:128, :], sn_t[0:64, :])
                nc.vector.tensor_mul(tmp[64:128, :], ps[0:64, :], sn_t[64:128, :])
                tmp2 = pr.tile([128, TB], BF16, tag="rcos", name=_nm("rcos"))
                nc.vector.tensor_mul(tmp2[:], ps[:], cs_t[:])
                nc.vector.tensor_add(dst, tmp[:], tmp2[:])

            wo_strips = {}

            def wo_prefetch(og):
                strip = pwo.tile([128, NHL, 256], BF16, tag="wo", name=_nm("wo"))
                nc.scalar.dma_start(out=strip[:], in_=wot_r[:, :, og * 256:(og + 1) * 256])
                wo_strips[og] = strip

            def wo_chunk(at_prev, tsl_prev, og, prefetch_next=True):
                strip = wo_strips.pop(og)
                if prefetch_next and og + 3 < H // 256:
                    wo_prefetch(og + 3)
                ob = pob.tile([128, 2, TB], BF16, tag="ob", name=_nm("ob"))
                for b in range(2):
                    ps = psA.tile([128, TB], F32, tag="A", name=_nm("psa"))
                    for hf in range(NHL):
                        nc.tensor.matmul(ps[:], strip[:, hf, b * 128:(b + 1) * 128],
                                         at_prev[hf][:], start=(hf == 0),
                                         stop=(hf == NHL - 1))
                    nc.vector.tensor_copy(ob[:, b, :], ps[:])
                    nc.scalar.dma_start(out=out_r[:, og, b, tsl_prev], in_=ob[:, b, :])

            prev_at = None
            prev_tsl = None

            for tb in range(NTB):
                tsl = slice(tb * TB, (tb + 1) * TB)
                qb = tb
                nkt = 4 * qb + 4  # k-tiles of 128 covering this q block

                cs_t = pcs.tile([128, TB], BF16, tag="cs", name=_nm("cs"))
                nc.sync.dma_start(out=cs_t[:], in_=cs[:, tsl])
                sn_t = pcs.tile([128, TB], BF16, tag="sn", name=_nm("sn"))
                nc.sync.dma_start(out=sn_t[:], in_=sn[:, tsl])

                xsingles = []
                xbigs = []
                if tb == 0:
                    # interleave K-weight chunks (both kv heads per chunk,
                    # 512B runs) with the x chunks in k-consumption order
                    kwt = pkw.tile([128, KB, KF], BF16, tag="kw", name=_nm("kw"))
                    nc.sync.dma_start(out=kwt[:, 0:8, :], in_=wkt_r[:, 0:8, :])
                    for c in range(4):
                        t = pxs.tile([128, TB], BF16, tag="xs", name=_nm("xs"))
                        nc.sync.dma_start(out=t[:], in_=xt[c * 128:(c + 1) * 128, tsl])
                        xsingles.append(t)
                    for g in range(1, 8):
                        t = px.tile([128, 4, TB], BF16, tag="xx", name=_nm("xx"))
                        nc.sync.dma_start(out=t[:], in_=xt_r[:, g, :, tsl])
                        xbigs.append((g, t))
                        if g in (2, 4, 6):
                            c = g // 2
                            nc.sync.dma_start(out=kwt[:, 8 * c:8 * c + 8, :],
                                              in_=wkt_r[:, 8 * c:8 * c + 8, :])
                else:
                    for g in range(0, 8):
                        t = px.tile([128, 4, TB], BF16, tag="xx", name=_nm("xx"))
                        nc.sync.dma_start(out=t[:], in_=xt_r[:, g, :, tsl])
                        xbigs.append((g, t))

                if tb == 0:
                    vstrip = pwv.tile([128, KB, KF], BF16, tag="wv", name=_nm("wv"))
                    for c in range(4):
                        nc.sync.dma_start(out=vstrip[:, 8 * c:8 * c + 8, :],
                                          in_=wvt_r[:, 8 * c:8 * c + 8, :])
                xmap = {}
                for k, t in enumerate(xsingles):
                    xmap[k] = t[:]
                for g, t in xbigs:
                    for b in range(4):
                        xmap[4 * g + b] = t[:, b, :]

                # Q weights load as head-pairs (512B DMA runs); pair 0 up front
                qpairs = {}

                def qpair_load(p):
                    s = pwqk.tile([128, KB, 256], BF16, tag="wqk", name=_nm("wqk"))
                    nc.sync.dma_start(out=s[:], in_=wqt_r[:, :, p * 256:(p + 1) * 256])
                    qpairs[p] = s

                qpair_load(0)

                # ---- K projection (kv head 0) + RoPE -> kts[0][:, tsl]
                def kproj(m):
                    ps = psA.tile([128, TB], F32, tag="A", name=_nm("psa"))
                    for k in range(KB):
                        nc.tensor.matmul(ps[:], kwt[:, k, m * 128:(m + 1) * 128], xmap[k],
                                         start=(k == 0), stop=(k == KB - 1))
                    rope(kts[m][:, tsl], ps, cs_t, sn_t)

                kproj(0)

                # ---- V projection -> v tiles [128, NKVL, 129]
                for t in range(4):
                    ps = psA.tile([128, TB], F32, tag="A", name=_nm("psa"))
                    for k in range(KB):
                        nc.tensor.matmul(ps[:, 0:KF], xmap[k][:, t * 128:(t + 1) * 128],
                                         vstrip[:, k, :], start=(k == 0), stop=(k == KB - 1))
                    vt = pv.tile([128, NKVL, 129], BF16, tag="vv", name=_nm("vv"))
                    for kv in range(NKVL):
                        nc.scalar.activation(vt[:, kv, 0:128],
                                             ps[:, kv * 128:(kv + 1) * 128], COPYF)
                    nc.vector.memset(vt[:, :, 128:129], 1.0)
                    vts.append(vt)

                # kv head 1's K projection (first needed at head 4)
                kproj(1)

                # ---- per head: Q proj + RoPE + scores; PV of previous head;
                # Wo chunks of the previous token block fill the pipeline
                at_tb = [pa.tile([128, TB], BF16, tag="at", name=_nm("at")) for _ in range(NHL)]

                def qproj(m):
                    p, half = m // 2, m % 2
                    if half == 0 and p + 1 < NHL // 2:
                        qpair_load(p + 1)
                    strip = qpairs[p]
                    ps = psA.tile([128, TB], F32, tag="A", name=_nm("psa"))
                    for k in range(KB):
                        nc.tensor.matmul(ps[:], strip[:, k, half * 128:(half + 1) * 128],
                                         xmap[k], start=(k == 0), stop=(k == KB - 1))
                    qd = pq.tile([128, TB], BF16, tag="qt", name=_nm("qt"))
                    rope(qd[:], ps, cs_t, sn_t)
                    if half == 1:
                        qpairs.pop(p)
                    return qd

                def st_sweep(h, qd):
                    kv = h // 4
                    pts = []
                    for kt in range(nkt):
                        lo = max(0, (kt - 4 * qb)) * 128  # causal trim
                        st = psS.tile([128, TB], F32, tag="st", name=_nm("st"))
                        nc.tensor.matmul(st[:, lo:TB], kts[kv][:, kt * 128:(kt + 1) * 128],
                                         qd[:, lo:TB], start=True, stop=True)
                        ptile = pp.tile([128, TB], BF16, tag="pt", name=_nm("pt"))
                        nc.scalar.activation(ptile[:, lo:TB], st[:, lo:TB], EXP,
                                             bias=0.0, scale=SCALE)
                        if kt >= 4 * qb:
                            # triangle mask only on the diagonal 128-chunk
                            nc.vector.tensor_mul(ptile[:, lo:lo + 128],
                                                 ptile[:, lo:lo + 128], msk_sb[:])
                        pts.append(ptile)
                    return pts

                def pv_sweep(h, pts):
                    kv = h // 4
                    pend = []

                    def emit_T(j, a_sb):
                        tr = psS.tile([128, 512], BF16, tag="st", name=_nm("tr"))
                        nc.tensor.transpose(tr[:, 0:128], a_sb[:], idn_sb[:])
                        nc.scalar.activation(at_tb[h][:, j * 128:(j + 1) * 128],
                                             tr[:, 0:128], COPYF)

                    for j in range(4):
                        nk = 4 * qb + j + 1
                        o = psO.tile([128, TB], F32, tag="o", name=_nm("o"))
                        for kt in range(nk):
                            nc.tensor.matmul(o[:, 0:129], pts[kt][:, j * 128:(j + 1) * 128],
                                             vts[kt][:, kv, :], start=(kt == 0),
                                             stop=(kt == nk - 1))
                        r = psmall.tile([128, 1], F32, tag="r", name=_nm("r"))
                        nc.vector.reciprocal(r[:], o[:, 128:129])
                        a_sb = pasb.tile([128, 128], BF16, tag="asb", name=_nm("asb"))
                        nc.vector.tensor_scalar_mul(a_sb[:], o[:, 0:128], r[:])
                        pend.append((j, a_sb))
                        if len(pend) > 2:
                            emit_T(*pend.pop(0))
                    return pend, emit_T

                pendT = None
                prev = None
                next_og = 0
                for h in range(NHL):
                    if h == 6:
                        # strips for this tb's Wo (runs during the next tb,
                        # or as the final tail for the last tb)
                        wo_prefetch(0)
                        wo_prefetch(1)
                        wo_prefetch(2)
                    qd = qproj(h)
                    if pendT is not None:
                        pend, emitter = pendT
                        for e in pend:
                            emitter(*e)
                        pendT = None
                    pts = st_sweep(h, qd)
                    if prev is not None:
                        pendT = pv_sweep(*prev)
                    prev = (h, pts)
                    if prev_at is not None:
                        wo_chunk(prev_at, prev_tsl, next_og)
                        next_og += 1
                        wo_chunk(prev_at, prev_tsl, next_og)
                        next_og += 1
                if pendT is not None:
                    pend, emitter = pendT
                    for e in pend:
                        emitter(*e)
                pend, emitter = pv_sweep(*prev)
                for e in pend:
                    emitter(*e)

                prev_at = at_tb
                prev_tsl = tsl

            # ---- final token block's Wo runs as the tail
            for og in range(H // 256):
                wo_chunk(prev_at, prev_tsl, og)

    nc.compile()
    _CACHE["nc"] = nc
    return nc


def _prep(hidden_states, Wq, Wk, Wv, Wo, position_ids):
    bf16 = ml_dtypes.bfloat16

    inv = 1.0 / (10000.0 ** (np.arange(0, HD, 2, dtype=np.float64) / HD))  # [64]
    kk = np.arange(128)[:, None]
    qq = np.arange(128)[None, :]
    mskc = (qq >= kk).astype(bf16)
    idnc = np.eye(128, dtype=np.float32).astype(bf16)

    in_maps = []
    for c in range(8):
        b, g = c // 4, c % 4
        xtn = np.ascontiguousarray(hidden_states[b].T).astype(bf16)
        wqtc = np.ascontiguousarray(Wq[QF * g:QF * (g + 1), :].T).astype(bf16)
        wktc = np.ascontiguousarray(Wk[KF * g:KF * (g + 1), :].T).astype(bf16)
        wvtc = np.ascontiguousarray(Wv[KF * g:KF * (g + 1), :].T).astype(bf16)
        wotc = np.ascontiguousarray(Wo[:, QF * g:QF * (g + 1)].T).astype(bf16)
        pos = position_ids[b].astype(np.float64)
        ang = inv[:, None] * pos[None, :]  # [64, S]
        cosf = np.concatenate([np.cos(ang), np.cos(ang)], 0).astype(bf16)
        sinb = np.sin(ang)
        sinf = np.concatenate([-sinb, sinb], 0).astype(bf16)
        in_maps.append(dict(xt=xtn, wqt=wqtc, wkt=wktc, wvt=wvtc, wot=wotc,
                            cs=cosf, sn=sinf, msk=mskc, idn=idnc))
    return in_maps


def _assemble(res, inputs):
    B = inputs["hidden_states"].shape[0]
    out = np.empty((B, S, H), np.float32)
    for b in range(B):
        acc = res.results[4 * b]["out_t"].astype(np.float32)
        for g in range(1, 4):
            acc = acc + res.results[4 * b + g]["out_t"].astype(np.float32)
        out[b] = acc.T
    return out


def kernel(hidden_states, Wq, Wk, Wv, Wo, position_ids):
    from concourse.bass_utils import run_bass_kernel_spmd

    hidden_states = np.asarray(hidden_states)
    Wq, Wk, Wv, Wo = (np.asarray(a) for a in (Wq, Wk, Wv, Wo))
    position_ids = np.asarray(position_ids)

    nc = _build()
    in_maps = _prep(hidden_states, Wq, Wk, Wv, Wo, position_ids)
    res = run_bass_kernel_spmd(nc, in_maps, list(range(8)))
    LAST["exec_time_ns"] = getattr(res, "exec_time_ns", None)

    return _assemble(res, dict(hidden_states=hidden_states))


# revision 42
# speedup vs baseline: 1.0320x; 1.0320x over previous
"""LlamaAttention (B=2,S=2048,H=4096, 32 q heads / 8 kv heads, RoPE, causal)
on 8 trn2 cores. Sharding: DP=2 over batch x TP=4 over heads.

v16: v15 + halved DMA packet counts for Wk/Wq (Wk one both-heads tile,
Wq head-pair strips -> 512B runs instead of 256B; the ramp is
packet-rate bound).
"""
import sys
if "/opt/trn_rl_repo" not in sys.path:
    sys.path.insert(0, "/opt/trn_rl_repo")

import numpy as np
import ml_dtypes

S = 2048
H = 4096
HD = 128
NHL = 8        # q heads per core
NKVL = 2       # kv heads per core
QF = NHL * HD  # 1024
KF = NKVL * HD  # 256
TB = 512       # token block
NTB = S // TB  # 4
KB = H // 128  # 32 contraction tiles for projections

_CACHE = {}
LAST = {}


def _build():
    if "nc" in _CACHE:
        return _CACHE["nc"]
    import concourse.bacc as bacc
    import concourse.mybir as mybir
    from concourse.tile import TileContext

    F32 = mybir.dt.float32
    BF16 = mybir.dt.bfloat16
    EXP = mybir.ActivationFunctionType.Exp
    COPYF = mybir.ActivationFunctionType.Copy
    SCALE = 1.0 / float(np.sqrt(HD))

    _ctr = [0]

    def _nm(p):
        _ctr[0] += 1
        return f"{p}{_ctr[0]}"

    nc = bacc.Bacc("TRN2", target_bir_lowering=False, debug=False, num_devices=8)
    xt = nc.declare_dram_parameter("xt", [H, S], BF16, isOutput=False)
    wqt = nc.declare_dram_parameter("wqt", [H, QF], BF16, isOutput=False)
    wkt = nc.declare_dram_parameter("wkt", [H, KF], BF16, isOutput=False)
    wvt = nc.declare_dram_parameter("wvt", [H, KF], BF16, isOutput=False)
    wot = nc.declare_dram_parameter("wot", [QF, H], BF16, isOutput=False)
    cs = nc.declare_dram_parameter("cs", [128, S], BF16, isOutput=False)
    sn = nc.declare_dram_parameter("sn", [128, S], BF16, isOutput=False)
    msk = nc.declare_dram_parameter("msk", [128, 128], BF16, isOutput=False)
    idn = nc.declare_dram_parameter("idn", [128, 128], BF16, isOutput=False)
    out_t = nc.declare_dram_parameter("out_t", [H, S], BF16, isOutput=True)

    wqt_r = wqt.rearrange("(kb p) m -> p kb m", p=128)
    wkt_r = wkt.rearrange("(kb p) m -> p kb m", p=128)
    wvt_r = wvt.rearrange("(kb p) m -> p kb m", p=128)
    wot_r = wot.rearrange("(kb p) m -> p kb m", p=128)
    xt_r = xt.rearrange("(kg b p) t -> p kg b t", p=128, b=4)  # [128, 8, 4, S]
    out_r = out_t.rearrange("(og b p) t -> p og b t", p=128, b=2)  # [128, 16, 2, S]

    from contextlib import ExitStack

    with ExitStack() as ctx:
        tc = ctx.enter_context(TileContext(nc))
        pc = ctx.enter_context(tc.tile_pool(name="const", bufs=1))
        px = ctx.enter_context(tc.tile_pool(name="xx", bufs=9))
        pxs = ctx.enter_context(tc.tile_pool(name="xs", bufs=4))
        pwqk = ctx.enter_context(tc.tile_pool(name="wqk", bufs=2))
        pkw = ctx.enter_context(tc.tile_pool(name="kw", bufs=1))
        pwv = ctx.enter_context(tc.tile_pool(name="wv", bufs=1))
        pq = ctx.enter_context(tc.tile_pool(name="qt", bufs=10))
        pk = ctx.enter_context(tc.tile_pool(name="kt", bufs=2))
        pv = ctx.enter_context(tc.tile_pool(name="vv", bufs=16))
        pa = ctx.enter_context(tc.tile_pool(name="at", bufs=16))
        pp = ctx.enter_context(tc.tile_pool(name="pt", bufs=26))
        pasb = ctx.enter_context(tc.tile_pool(name="asb", bufs=10))
        pcs = ctx.enter_context(tc.tile_pool(name="csn", bufs=2))
        pr = ctx.enter_context(tc.tile_pool(name="rope", bufs=2))
        psmall = ctx.enter_context(tc.tile_pool(name="sm", bufs=8))
        pwo = ctx.enter_context(tc.tile_pool(name="wo", bufs=4))
        pob = ctx.enter_context(tc.tile_pool(name="ob", bufs=3))
        psA = ctx.enter_context(tc.tile_pool(name="psA", bufs=2, space="PSUM"))
        psS = ctx.enter_context(tc.tile_pool(name="psS", bufs=3, space="PSUM"))
        psO = ctx.enter_context(tc.tile_pool(name="psO", bufs=3, space="PSUM"))
        if True:
            msk_sb = pc.tile([128, 128], BF16, tag="msk")
            nc.sync.dma_start(out=msk_sb[:], in_=msk[:])
            idn_sb = pc.tile([128, 128], BF16, tag="idn")
            nc.sync.dma_start(out=idn_sb[:], in_=idn[:])

            # HAM warmup: dependency-free matmuls on the identity tile keep
            # the PE busy through a SHORT window so real chains start at 2.4G.
            for _ in range(28):
                w = psS.tile([128, 512], F32, tag="st", name=_nm("warm"))
                nc.tensor.matmul(w[:, 0:128], idn_sb[:], idn_sb[:],
                                 start=True, stop=True)

            # persistent K^T [hd, S] per kv head, and V [tok, (kv, hd|1)]
            kts = [pk.tile([128, S], BF16, tag="kt", name=f"ktp{i}") for i in range(NKVL)]
            vts = []  # 16 tiles [128, NKVL, 129]
            kwt = None
            vstrip = None

            def rope(dst, ps, cs_t, sn_t):
                tmp = pr.tile([128, TB], BF16, tag="rsin", name=_nm("rsin"))
                nc.vector.tensor_mul(tmp[0:64, :], ps[64:128, :], sn_t[0:64, :])
                nc.vector.tensor_mul(tmp[64:128, :], ps[0:64, :], sn_t[64:128, :])
                tmp2 = pr.tile([128, TB], BF16, tag="rcos", name=_nm("rcos"))
                nc.vector.tensor_mul(tmp2[:], ps[:], cs_t[:])
                nc.vector.tensor_add(dst, tmp[:], tmp2[:])

            wo_strips = {}

            def wo_prefetch(og):
                strip = pwo.tile([128, NHL, 256], BF16, tag="wo", name=_nm("wo"))
                nc.scalar.dma_start(out=strip[:], in_=wot_r[:, :, og * 256:(og + 1) * 256])
                wo_strips[og] = strip

            def wo_chunk(at_prev, tsl_prev, og, prefetch_next=True):
                strip = wo_strips.pop(og)
                if prefetch_next and og + 2 < H // 256:
                    wo_prefetch(og + 2)
                ob = pob.tile([128, 2, TB], BF16, tag="ob", name=_nm("ob"))
                for b in range(2):
                    ps = psA.tile([128, TB], F32, tag="A", name=_nm("psa"))
                    for hf in range(NHL):
                        nc.tensor.matmul(ps[:], strip[:, hf, b * 128:(b + 1) * 128],
                                         at_prev[hf][:], start=(hf == 0),
                                         stop=(hf == NHL - 1))
                    nc.vector.tensor_copy(ob[:, b, :], ps[:])
                    nc.scalar.dma_start(out=out_r[:, og, b, tsl_prev], in_=ob[:, b, :])

            prev_at = None
            prev_tsl = None

            for tb in range(NTB):
                tsl = slice(tb * TB, (tb + 1) * TB)
                qb = tb
                nkt = 4 * qb + 4  # k-tiles of 128 covering this q block

                xsingles = []
                xbigs = []
                if tb == 0:
                    # interleave K-weight chunks (both kv heads per chunk,
                    # 512B runs) with the x chunks in k-consumption order
                    kwt = pkw.tile([128, KB, KF], BF16, tag="kw", name=_nm("kw"))
                    nc.sync.dma_start(out=kwt[:, 0:8, :], in_=wkt_r[:, 0:8, :])
                    for c in range(4):
                        t = pxs.tile([128, TB], BF16, tag="xs", name=_nm("xs"))
                        nc.sync.dma_start(out=t[:], in_=xt[c * 128:(c + 1) * 128, tsl])
                        xsingles.append(t)
                    for g in range(1, 8):
                        t = px.tile([128, 4, TB], BF16, tag="xx", name=_nm("xx"))
                        nc.sync.dma_start(out=t[:], in_=xt_r[:, g, :, tsl])
                        xbigs.append((g, t))
                        if g in (2, 4, 6):
                            c = g // 2
                            nc.sync.dma_start(out=kwt[:, 8 * c:8 * c + 8, :],
                                              in_=wkt_r[:, 8 * c:8 * c + 8, :])
                else:
                    for g in range(0, 8):
                        t = px.tile([128, 4, TB], BF16, tag="xx", name=_nm("xx"))
                        nc.sync.dma_start(out=t[:], in_=xt_r[:, g, :, tsl])
                        xbigs.append((g, t))

                cs_t = pcs.tile([128, TB], BF16, tag="cs", name=_nm("cs"))
                nc.sync.dma_start(out=cs_t[:], in_=cs[:, tsl])
                sn_t = pcs.tile([128, TB], BF16, tag="sn", name=_nm("sn"))
                nc.sync.dma_start(out=sn_t[:], in_=sn[:, tsl])

                if tb == 0:
                    vstrip = pwv.tile([128, KB, KF], BF16, tag="wv", name=_nm("wv"))
                    for c in range(4):
                        nc.sync.dma_start(out=vstrip[:, 8 * c:8 * c + 8, :],
                                          in_=wvt_r[:, 8 * c:8 * c + 8, :])
                xmap = {}
                for k, t in enumerate(xsingles):
                    xmap[k] = t[:]
                for g, t in xbigs:
                    for b in range(4):
                        xmap[4 * g + b] = t[:, b, :]

                # Q weights load as head-pairs (512B DMA runs); pair 0 up front
                qpairs = {}

                def qpair_load(p):
                    s = pwqk.tile([128, KB, 256], BF16, tag="wqk", name=_nm("wqk"))
                    nc.sync.dma_start(out=s[:], in_=wqt_r[:, :, p * 256:(p + 1) * 256])
                    qpairs[p] = s

                qpair_load(0)

                # ---- K projection + RoPE -> kts[m][:, tsl]
                for m in range(NKVL):
                    ps = psA.tile([128, TB], F32, tag="A", name=_nm("psa"))
                    for k in range(KB):
                        nc.tensor.matmul(ps[:], kwt[:, k, m * 128:(m + 1) * 128], xmap[k],
                                         start=(k == 0), stop=(k == KB - 1))
                    rope(kts[m][:, tsl], ps, cs_t, sn_t)

                # ---- V projection -> v tiles [128, NKVL, 129]
                for t in range(4):
                    ps = psA.tile([128, TB], F32, tag="A", name=_nm("psa"))
                    for k in range(KB):
                        nc.tensor.matmul(ps[:, 0:KF], xmap[k][:, t * 128:(t + 1) * 128],
                                         vstrip[:, k, :], start=(k == 0), stop=(k == KB - 1))
                    vt = pv.tile([128, NKVL, 129], BF16, tag="vv", name=_nm("vv"))
                    for kv in range(NKVL):
                        nc.scalar.activation(vt[:, kv, 0:128],
                                             ps[:, kv * 128:(kv + 1) * 128], COPYF)
                    nc.vector.memset(vt[:, :, 128:129], 1.0)
                    vts.append(vt)

                # ---- per head: Q proj + RoPE + scores; PV of previous head;
                # Wo chunks of the previous token block fill the pipeline
                at_tb = [pa.tile([128, TB], BF16, tag="at", name=_nm("at")) for _ in range(NHL)]

                def qproj(m):
                    p, half = m // 2, m % 2
                    if half == 0 and p + 1 < NHL // 2:
                        qpair_load(p + 1)
                    strip = qpairs[p]
                    ps = psA.tile([128, TB], F32, tag="A", name=_nm("psa"))
                    for k in range(KB):
                        nc.tensor.matmul(ps[:], strip[:, k, half * 128:(half + 1) * 128],
                                         xmap[k], start=(k == 0), stop=(k == KB - 1))
                    qd = pq.tile([128, TB], BF16, tag="qt", name=_nm("qt"))
                    rope(qd[:], ps, cs_t, sn_t)
                    if half == 1:
                        qpairs.pop(p)
                    return qd

                def st_sweep(h, qd):
                    kv = h // 4
                    pts = []
                    for kt in range(nkt):
                        lo = max(0, (kt - 4 * qb)) * 128  # causal trim
                        st = psS.tile([128, TB], F32, tag="st", name=_nm("st"))
                        nc.tensor.matmul(st[:, lo:TB], kts[kv][:, kt * 128:(kt + 1) * 128],
                                         qd[:, lo:TB], start=True, stop=True)
                        ptile = pp.tile([128, TB], BF16, tag="pt", name=_nm("pt"))
                        nc.scalar.activation(ptile[:, lo:TB], st[:, lo:TB], EXP,
                                             bias=0.0, scale=SCALE)
                        if kt >= 4 * qb:
                            # triangle mask only on the diagonal 128-chunk
                            nc.vector.tensor_mul(ptile[:, lo:lo + 128],
                                                 ptile[:, lo:lo + 128], msk_sb[:])
                        pts.append(ptile)
                    return pts

                def pv_sweep(h, pts):
                    kv = h // 4
                    pend = []

                    def emit_T(j, a_sb):
                        tr = psS.tile([128, 512], BF16, tag="st", name=_nm("tr"))
                        nc.tensor.transpose(tr[:, 0:128], a_sb[:], idn_sb[:])
                        nc.scalar.activation(at_tb[h][:, j * 128:(j + 1) * 128],
                                             tr[:, 0:128], COPYF)

                    for j in range(4):
                        nk = 4 * qb + j + 1
                        o = psO.tile([128, TB], F32, tag="o", name=_nm("o"))
                        for kt in range(nk):
                            nc.tensor.matmul(o[:, 0:129], pts[kt][:, j * 128:(j + 1) * 128],
                                             vts[kt][:, kv, :], start=(kt == 0),
                                             stop=(kt == nk - 1))
                        r = psmall.tile([128, 1], F32, tag="r", name=_nm("r"))
                        nc.vector.reciprocal(r[:], o[:, 128:129])
                        a_sb = pasb.tile([128, 128], BF16, tag="asb", name=_nm("asb"))
                        nc.vector.tensor_scalar_mul(a_sb[:], o[:, 0:128], r[:])
                        pend.append((j, a_sb))
                        if len(pend) > 2:
                            emit_T(*pend.pop(0))
                    return pend, emit_T

                pendT = None
                prev = None
                next_og = 0
                for h in range(NHL):
                    if h == 6:
                        # strips for this tb's Wo (runs during the next tb,
                        # or as the final tail for the last tb)
                        wo_prefetch(0)
                        wo_prefetch(1)
                    qd = qproj(h)
                    if pendT is not None:
                        pend, emitter = pendT
                        for e in pend:
                            emitter(*e)
                        pendT = None
                    pts = st_sweep(h, qd)
                    if prev is not None:
                        pendT = pv_sweep(*prev)
                    prev = (h, pts)
                    if prev_at is not None:
                        wo_chunk(prev_at, prev_tsl, next_og)
                        next_og += 1
                        wo_chunk(prev_at, prev_tsl, next_og)
                        next_og += 1
                if pendT is not None:
                    pend, emitter = pendT
                    for e in pend:
                        emitter(*e)
                pend, emitter = pv_sweep(*prev)
                for e in pend:
                    emitter(*e)

                prev_at = at_tb
                prev_tsl = tsl

            # ---- final token block's Wo runs as the tail
            for og in range(H // 256):
                wo_chunk(prev_at, prev_tsl, og)

    nc.compile()
    _CACHE["nc"] = nc
    return nc


def _prep(hidden_states, Wq, Wk, Wv, Wo, position_ids):
    bf16 = ml_dtypes.bfloat16

    inv = 1.0 / (10000.0 ** (np.arange(0, HD, 2, dtype=np.float64) / HD))  # [64]
    kk = np.arange(128)[:, None]
    qq = np.arange(128)[None, :]
    mskc = (qq >= kk).astype(bf16)
    idnc = np.eye(128, dtype=np.float32).astype(bf16)

    in_maps = []
    for c in range(8):
        b, g = c // 4, c % 4
        xtn = np.ascontiguousarray(hidden_states[b].T).astype(bf16)
        wqtc = np.ascontiguousarray(Wq[QF * g:QF * (g + 1), :].T).astype(bf16)
        wktc = np.ascontiguousarray(Wk[KF * g:KF * (g + 1), :].T).astype(bf16)
        wvtc = np.ascontiguousarray(Wv[KF * g:KF * (g + 1), :].T).astype(bf16)
        wotc = np.ascontiguousarray(Wo[:, QF * g:QF * (g + 1)].T).astype(bf16)
        pos = position_ids[b].astype(np.float64)
        ang = inv[:, None] * pos[None, :]  # [64, S]
        cosf = np.concatenate([np.cos(ang), np.cos(ang)], 0).astype(bf16)
        sinb = np.sin(ang)
        sinf = np.concatenate([-sinb, sinb], 0).astype(bf16)
        in_maps.append(dict(xt=xtn, wqt=wqtc, wkt=wktc, wvt=wvtc, wot=wotc,
                            cs=cosf, sn=sinf, msk=mskc, idn=idnc))
    return in_maps


def _assemble(res, inputs):
    B = inputs["hidden_states"].shape[0]
    out = np.empty((B, S, H), np.float32)
    for b in range(B):
        acc = res.results[4 * b]["out_t"].astype(np.float32)
        for g in range(1, 4):
            acc = acc + res.results[4 * b + g]["out_t"].astype(np.float32)
        out[b] = acc.T
    return out


def kernel(hidden_states, Wq, Wk, Wv, Wo, position_ids):
    from concourse.bass_utils import run_bass_kernel_spmd

    hidden_states = np.asarray(hidden_states)
    Wq, Wk, Wv, Wo = (np.asarray(a) for a in (Wq, Wk, Wv, Wo))
    position_ids = np.asarray(position_ids)

    nc = _build()
    in_maps = _prep(hidden_states, Wq, Wk, Wv, Wo, position_ids)
    res = run_bass_kernel_spmd(nc, in_maps, list(range(8)))
    LAST["exec_time_ns"] = getattr(res, "exec_time_ns", None)

    return _assemble(res, dict(hidden_states=hidden_states))


# revision 43
# speedup vs baseline: 1.0442x; 1.0117x over previous
"""LlamaAttention (B=2,S=2048,H=4096, 32 q heads / 8 kv heads, RoPE, causal)
on 8 trn2 cores. Sharding: DP=2 over batch x TP=4 over heads.

v19: v16 + partition-major DRAM layouts prepared on the host: every
x/weight DMA now reads 4-16KB contiguous per partition (the ramp was
DMA packet-rate bound at ~500 pkts/us).
"""
import sys
if "/opt/trn_rl_repo" not in sys.path:
    sys.path.insert(0, "/opt/trn_rl_repo")

import numpy as np
import ml_dtypes

S = 2048
H = 4096
HD = 128
NHL = 8        # q heads per core
NKVL = 2       # kv heads per core
QF = NHL * HD  # 1024
KF = NKVL * HD  # 256
TB = 512       # token block
NTB = S // TB  # 4
KB = H // 128  # 32 contraction tiles for projections

_CACHE = {}
LAST = {}


def _build():
    if "nc" in _CACHE:
        return _CACHE["nc"]
    import concourse.bacc as bacc
    import concourse.mybir as mybir
    from concourse.tile import TileContext

    F32 = mybir.dt.float32
    BF16 = mybir.dt.bfloat16
    EXP = mybir.ActivationFunctionType.Exp
    COPYF = mybir.ActivationFunctionType.Copy
    SCALE = 1.0 / float(np.sqrt(HD))

    _ctr = [0]

    def _nm(p):
        _ctr[0] += 1
        return f"{p}{_ctr[0]}"

    nc = bacc.Bacc("TRN2", target_bir_lowering=False, debug=False, num_devices=8)
    # partition-major layouts: per-partition reads are long contiguous runs
    xq = nc.declare_dram_parameter("xq", [8, 128, NTB, 4, TB], BF16, isOutput=False)
    wqp = nc.declare_dram_parameter("wqp", [4, 128, KB, 256], BF16, isOutput=False)
    wkp = nc.declare_dram_parameter("wkp", [128, KB, KF], BF16, isOutput=False)
    wvp = nc.declare_dram_parameter("wvp", [128, KB, KF], BF16, isOutput=False)
    wop = nc.declare_dram_parameter("wop", [H // 256, 128, NHL, 256], BF16, isOutput=False)
    cs = nc.declare_dram_parameter("cs", [128, S], BF16, isOutput=False)
    sn = nc.declare_dram_parameter("sn", [128, S], BF16, isOutput=False)
    msk = nc.declare_dram_parameter("msk", [128, 128], BF16, isOutput=False)
    idn = nc.declare_dram_parameter("idn", [128, 128], BF16, isOutput=False)
    out_t = nc.declare_dram_parameter("out_t", [H, S], BF16, isOutput=True)

    out_r = out_t.rearrange("(og b p) t -> p og b t", p=128, b=2)  # [128, 16, 2, S]

    from contextlib import ExitStack

    with ExitStack() as ctx:
        tc = ctx.enter_context(TileContext(nc))
        pc = ctx.enter_context(tc.tile_pool(name="const", bufs=1))
        px = ctx.enter_context(tc.tile_pool(name="xx", bufs=9))
        pxs = ctx.enter_context(tc.tile_pool(name="xs", bufs=4))
        pwqk = ctx.enter_context(tc.tile_pool(name="wqk", bufs=2))
        pkw = ctx.enter_context(tc.tile_pool(name="kw", bufs=1))
        pwv = ctx.enter_context(tc.tile_pool(name="wv", bufs=1))
        pq = ctx.enter_context(tc.tile_pool(name="qt", bufs=10))
        pk = ctx.enter_context(tc.tile_pool(name="kt", bufs=2))
        pv = ctx.enter_context(tc.tile_pool(name="vv", bufs=16))
        pa = ctx.enter_context(tc.tile_pool(name="at", bufs=16))
        pp = ctx.enter_context(tc.tile_pool(name="pt", bufs=26))
        pasb = ctx.enter_context(tc.tile_pool(name="asb", bufs=10))
        pcs = ctx.enter_context(tc.tile_pool(name="csn", bufs=2))
        pr = ctx.enter_context(tc.tile_pool(name="rope", bufs=2))
        psmall = ctx.enter_context(tc.tile_pool(name="sm", bufs=8))
        pwo = ctx.enter_context(tc.tile_pool(name="wo", bufs=4))
        pob = ctx.enter_context(tc.tile_pool(name="ob", bufs=3))
        psA = ctx.enter_context(tc.tile_pool(name="psA", bufs=2, space="PSUM"))
        psS = ctx.enter_context(tc.tile_pool(name="psS", bufs=3, space="PSUM"))
        psO = ctx.enter_context(tc.tile_pool(name="psO", bufs=3, space="PSUM"))
        if True:
            msk_sb = pc.tile([128, 128], BF16, tag="msk")
            nc.sync.dma_start(out=msk_sb[:], in_=msk[:])
            idn_sb = pc.tile([128, 128], BF16, tag="idn")
            nc.sync.dma_start(out=idn_sb[:], in_=idn[:])

            # HAM warmup: dependency-free matmuls on the identity tile keep
            # the PE busy through a SHORT window so real chains start at 2.4G.
            for _ in range(28):
                w = psS.tile([128, 512], F32, tag="st", name=_nm("warm"))
                nc.tensor.matmul(w[:, 0:128], idn_sb[:], idn_sb[:],
                                 start=True, stop=True)

            # persistent K^T [hd, S] per kv head, and V [tok, (kv, hd|1)]
            kts = [pk.tile([128, S], BF16, tag="kt", name=f"ktp{i}") for i in range(NKVL)]
            vts = []  # 16 tiles [128, NKVL, 129]
            kwt = None
            vstrip = None

            def rope(dst, ps, cs_t, sn_t):
                tmp = pr.tile([128, TB], BF16, tag="rsin", name=_nm("rsin"))
                nc.vector.tensor_mul(tmp[0:64, :], ps[64:128, :], sn_t[0:64, :])
                nc.vector.tensor_mul(tmp[64:128, :], ps[0:64, :], sn_t[64:128, :])
                tmp2 = pr.tile([128, TB], BF16, tag="rcos", name=_nm("rcos"))
                nc.vector.tensor_mul(tmp2[:], ps[:], cs_t[:])
                nc.vector.tensor_add(dst, tmp[:], tmp2[:])

            wo_strips = {}

            def wo_prefetch(og):
                strip = pwo.tile([128, NHL, 256], BF16, tag="wo", name=_nm("wo"))
                nc.scalar.dma_start(out=strip[:], in_=wop[og])
                wo_strips[og] = strip

            def wo_chunk(at_prev, tsl_prev, og, prefetch_next=True):
                strip = wo_strips.pop(og)
                if prefetch_next and og + 2 < H // 256:
                    wo_prefetch(og + 2)
                ob = pob.tile([128, 2, TB], BF16, tag="ob", name=_nm("ob"))
                for b in range(2):
                    ps = psA.tile([128, TB], F32, tag="A", name=_nm("psa"))
                    for hf in range(NHL):
                        nc.tensor.matmul(ps[:], strip[:, hf, b * 128:(b + 1) * 128],
                                         at_prev[hf][:], start=(hf == 0),
                                         stop=(hf == NHL - 1))
                    nc.vector.tensor_copy(ob[:, b, :], ps[:])
                    nc.scalar.dma_start(out=out_r[:, og, b, tsl_prev], in_=ob[:, b, :])

            prev_at = None
            prev_tsl = None

            for tb in range(NTB):
                tsl = slice(tb * TB, (tb + 1) * TB)
                qb = tb
                nkt = 4 * qb + 4  # k-tiles of 128 covering this q block

                xsingles = []
                xbigs = []
                if tb == 0:
                    # interleave K-weight chunks (both kv heads per chunk,
                    # 512B runs) with the x chunks in k-consumption order
                    kwt = pkw.tile([128, KB, KF], BF16, tag="kw", name=_nm("kw"))
                    nc.sync.dma_start(out=kwt[:, 0:8, :], in_=wkp[:, 0:8, :])
                    for c in range(4):
                        t = pxs.tile([128, TB], BF16, tag="xs", name=_nm("xs"))
                        nc.sync.dma_start(out=t[:], in_=xq[0, :, 0, c, :])
                        xsingles.append(t)
                    for g in range(1, 8):
                        t = px.tile([128, 4, TB], BF16, tag="xx", name=_nm("xx"))
                        nc.sync.dma_start(out=t[:], in_=xq[g, :, tb, :, :])
                        xbigs.append((g, t))
                        if g in (2, 4, 6):
                            c = g // 2
                            nc.sync.dma_start(out=kwt[:, 8 * c:8 * c + 8, :],
                                              in_=wkp[:, 8 * c:8 * c + 8, :])
                else:
                    for g in range(0, 8):
                        t = px.tile([128, 4, TB], BF16, tag="xx", name=_nm("xx"))
                        nc.sync.dma_start(out=t[:], in_=xq[g, :, tb, :, :])
                        xbigs.append((g, t))

                cs_t = pcs.tile([128, TB], BF16, tag="cs", name=_nm("cs"))
                nc.sync.dma_start(out=cs_t[:], in_=cs[:, tsl])
                sn_t = pcs.tile([128, TB], BF16, tag="sn", name=_nm("sn"))
                nc.sync.dma_start(out=sn_t[:], in_=sn[:, tsl])

                if tb == 0:
                    vstrip = pwv.tile([128, KB, KF], BF16, tag="wv", name=_nm("wv"))
                    for c in range(4):
                        nc.sync.dma_start(out=vstrip[:, 8 * c:8 * c + 8, :],
                                          in_=wvp[:, 8 * c:8 * c + 8, :])
                xmap = {}
                for k, t in enumerate(xsingles):
                    xmap[k] = t[:]
                for g, t in xbigs:
                    for b in range(4):
                        xmap[4 * g + b] = t[:, b, :]

                # Q weights load as head-pairs (512B DMA runs); pair 0 up front
                qpairs = {}

                def qpair_load(p):
                    s = pwqk.tile([128, KB, 256], BF16, tag="wqk", name=_nm("wqk"))
                    nc.sync.dma_start(out=s[:], in_=wqp[p])
                    qpairs[p] = s

                qpair_load(0)

                # ---- K projection + RoPE -> kts[m][:, tsl]
                for m in range(NKVL):
                    ps = psA.tile([128, TB], F32, tag="A", name=_nm("psa"))
                    for k in range(KB):
                        nc.tensor.matmul(ps[:], kwt[:, k, m * 128:(m + 1) * 128], xmap[k],
                                         start=(k == 0), stop=(k == KB - 1))
                    rope(kts[m][:, tsl], ps, cs_t, sn_t)

                # ---- V projection -> v tiles [128, NKVL, 129]
                for t in range(4):
                    ps = psA.tile([128, TB], F32, tag="A", name=_nm("psa"))
                    for k in range(KB):
                        nc.tensor.matmul(ps[:, 0:KF], xmap[k][:, t * 128:(t + 1) * 128],
                                         vstrip[:, k, :], start=(k == 0), stop=(k == KB - 1))
                    vt = pv.tile([128, NKVL, 129], BF16, tag="vv", name=_nm("vv"))
                    for kv in range(NKVL):
                        nc.scalar.activation(vt[:, kv, 0:128],
                                             ps[:, kv * 128:(kv + 1) * 128], COPYF)
                    nc.vector.memset(vt[:, :, 128:129], 1.0)
                    vts.append(vt)

                # ---- per head: Q proj + RoPE + scores; PV of previous head;
                # Wo chunks of the previous token block fill the pipeline
                at_tb = [pa.tile([128, TB], BF16, tag="at", name=_nm("at")) for _ in range(NHL)]

                def qproj(m):
                    p, half = m // 2, m % 2
                    if half == 0 and p + 1 < NHL // 2:
                        qpair_load(p + 1)
                    strip = qpairs[p]
                    ps = psA.tile([128, TB], F32, tag="A", name=_nm("psa"))
                    for k in range(KB):
                        nc.tensor.matmul(ps[:], strip[:, k, half * 128:(half + 1) * 128],
                                         xmap[k], start=(k == 0), stop=(k == KB - 1))
                    qd = pq.tile([128, TB], BF16, tag="qt", name=_nm("qt"))
                    rope(qd[:], ps, cs_t, sn_t)
                    if half == 1:
                        qpairs.pop(p)
                    return qd

                def st_sweep(h, qd):
                    kv = h // 4
                    pts = []
                    for kt in range(nkt):
                        lo = max(0, (kt - 4 * qb)) * 128  # causal trim
                        st = psS.tile([128, TB], F32, tag="st", name=_nm("st"))
                        nc.tensor.matmul(st[:, lo:TB], kts[kv][:, kt * 128:(kt + 1) * 128],
                                         qd[:, lo:TB], start=True, stop=True)
                        ptile = pp.tile([128, TB], BF16, tag="pt", name=_nm("pt"))
                        nc.scalar.activation(ptile[:, lo:TB], st[:, lo:TB], EXP,
                                             bias=0.0, scale=SCALE)
                        if kt >= 4 * qb:
                            # triangle mask only on the diagonal 128-chunk
                            nc.vector.tensor_mul(ptile[:, lo:lo + 128],
                                                 ptile[:, lo:lo + 128], msk_sb[:])
                        pts.append(ptile)
                    return pts

                def pv_sweep(h, pts):
                    kv = h // 4
                    pend = []

                    def emit_T(j, a_sb):
                        tr = psS.tile([128, 512], BF16, tag="st", name=_nm("tr"))
                        nc.tensor.transpose(tr[:, 0:128], a_sb[:], idn_sb[:])
                        nc.scalar.activation(at_tb[h][:, j * 128:(j + 1) * 128],
                                             tr[:, 0:128], COPYF)

                    for j in range(4):
                        nk = 4 * qb + j + 1
                        o = psO.tile([128, TB], F32, tag="o", name=_nm("o"))
                        for kt in range(nk):
                            nc.tensor.matmul(o[:, 0:129], pts[kt][:, j * 128:(j + 1) * 128],
                                             vts[kt][:, kv, :], start=(kt == 0),
                                             stop=(kt == nk - 1))
                        r = psmall.tile([128, 1], F32, tag="r", name=_nm("r"))
                        nc.vector.reciprocal(r[:], o[:, 128:129])
                        a_sb = pasb.tile([128, 128], BF16, tag="asb", name=_nm("asb"))
                        nc.vector.tensor_scalar_mul(a_sb[:], o[:, 0:128], r[:])
                        pend.append((j, a_sb))
                        if len(pend) > 2:
                            emit_T(*pend.pop(0))
                    return pend, emit_T

                pendT = None
                prev = None
                next_og = 0
                for h in range(NHL):
                    if h == 6:
                        # strips for this tb's Wo (runs during the next tb,
                        # or as the final tail for the last tb)
                        wo_prefetch(0)
                        wo_prefetch(1)
                    qd = qproj(h)
                    if pendT is not None:
                        pend, emitter = pendT
                        for e in pend:
                            emitter(*e)
                        pendT = None
                    pts = st_sweep(h, qd)
                    if prev is not None:
                        pendT = pv_sweep(*prev)
                    prev = (h, pts)
                    if prev_at is not None:
                        wo_chunk(prev_at, prev_tsl, next_og)
                        next_og += 1
                        wo_chunk(prev_at, prev_tsl, next_og)
                        next_og += 1
                if pendT is not None:
                    pend, emitter = pendT
                    for e in pend:
                        emitter(*e)
                pend, emitter = pv_sweep(*prev)
                for e in pend:
                    emitter(*e)

                prev_at = at_tb
                prev_tsl = tsl

            # ---- final token block's Wo runs as the tail
            for og in range(H // 256):
                wo_chunk(prev_at, prev_tsl, og)

    nc.compile()
    _CACHE["nc"] = nc
    return nc


def _prep(hidden_states, Wq, Wk, Wv, Wo, position_ids):
    bf16 = ml_dtypes.bfloat16

    inv = 1.0 / (10000.0 ** (np.arange(0, HD, 2, dtype=np.float64) / HD))  # [64]
    kk = np.arange(128)[:, None]
    qq = np.arange(128)[None, :]
    mskc = (qq >= kk).astype(bf16)
    idnc = np.eye(128, dtype=np.float32).astype(bf16)

    in_maps = []
    for c in range(8):
        b, g = c // 4, c % 4
        xtn = np.ascontiguousarray(hidden_states[b].T).astype(bf16)
        # partition-major images matching the kernel's DRAM declarations
        xqc = np.ascontiguousarray(
            xtn.reshape(8, 4, 128, NTB, TB).transpose(0, 2, 3, 1, 4))
        wqtc = np.ascontiguousarray(Wq[QF * g:QF * (g + 1), :].T).astype(bf16)
        wqpc = np.ascontiguousarray(
            wqtc.reshape(KB, 128, 4, 256).transpose(2, 1, 0, 3))
        wktc = np.ascontiguousarray(Wk[KF * g:KF * (g + 1), :].T).astype(bf16)
        wkpc = np.ascontiguousarray(wktc.reshape(KB, 128, KF).transpose(1, 0, 2))
        wvtc = np.ascontiguousarray(Wv[KF * g:KF * (g + 1), :].T).astype(bf16)
        wvpc = np.ascontiguousarray(wvtc.reshape(KB, 128, KF).transpose(1, 0, 2))
        wotc = np.ascontiguousarray(Wo[:, QF * g:QF * (g + 1)].T).astype(bf16)
        wopc = np.ascontiguousarray(
            wotc.reshape(NHL, 128, H // 256, 256).transpose(2, 1, 0, 3))
        pos = position_ids[b].astype(np.float64)
        ang = inv[:, None] * pos[None, :]  # [64, S]
        cosf = np.concatenate([np.cos(ang), np.cos(ang)], 0).astype(bf16)
        sinb = np.sin(ang)
        sinf = np.concatenate([-sinb, sinb], 0).astype(bf16)
        in_maps.append(dict(xq=xqc, wqp=wqpc, wkp=wkpc, wvp=wvpc, wop=wopc,
                            cs=cosf, sn=sinf, msk=mskc, idn=idnc))
    return in_maps


def _assemble(res, inputs):
    B = inputs["hidden_states"].shape[0]
    out = np.empty((B, S, H), np.float32)
    for b in range(B):
        acc = res.results[4 * b]["out_t"].astype(np.float32)
        for g in range(1, 4):
            acc = acc + res.results[4 * b + g]["out_t"].astype(np.float32)
        out[b] = acc.T
    return out


def kernel(hidden_states, Wq, Wk, Wv, Wo, position_ids):
    from concourse.bass_utils import run_bass_kernel_spmd

    hidden_states = np.asarray(hidden_states)
    Wq, Wk, Wv, Wo = (np.asarray(a) for a in (Wq, Wk, Wv, Wo))
    position_ids = np.asarray(position_ids)

    nc = _build()
    in_maps = _prep(hidden_states, Wq, Wk, Wv, Wo, position_ids)
    res = run_bass_kernel_spmd(nc, in_maps, list(range(8)))
    LAST["exec_time_ns"] = getattr(res, "exec_time_ns", None)

    return _assemble(res, dict(hidden_states=hidden_states))
